# revision 5
# baseline (speedup 1.0000x reference)
"""Trainium2 Bass kernel for MultiHeadSelfAttention (B=4, L=2048, H=1024, NH=16).

Sharding: 8 cores = 4 batches x 2 head-groups (8 heads each).

v2 structure (single pass, all pools coexist so the tile scheduler can
overlap phases):
  - QKV projections per head-pair, interleaved with the previous pair's
    attention so ScalarE/VectorE never idle behind TensorE.
  - Attention per pair: the two heads' S^T matmuls (K=64 contraction)
    are emitted back-to-back with tile_position (0,0)/(64,0) so the PE
    array runs them concurrently (row tiling).
  - Head A's AV runs inline; head B's masked probabilities are buffered
    in SBUF and its AV sweep runs afterwards, halving live PSUM demand
    (8 banks total: 2 qkv + 4 s + 2 nd).
  - Softmax denominator via 64 ones-columns augmented into v (numerator
    and denominator from one matmul).
  - exp on ScalarE (fused 1/8 scale), mask multiply + PSUM copies +
    reciprocal on VectorE; mask tiles streamed from DRAM per pair.
"""

import os
import sys
import contextlib

os.environ.setdefault("JAX_PLATFORMS", "")
try:
    import concourse.bass as bass  # noqa: F401
except ImportError:
    sys.path.insert(0, "/opt/trn_rl_repo")

import numpy as np
import ml_dtypes

import concourse.bass as bass
import concourse.mybir as mybir
import concourse.tile as tile
from concourse import bacc
from concourse import bass_utils

BF16 = mybir.dt.bfloat16
F32 = mybir.dt.float32

B, L, H = 4, 2048, 1024
NH, HD = 16, 64
NCORES = 8
HPC = NH // 2          # heads per core = 8
CPC = H // 2           # channels per core = 512
KAUG = 1152            # 1024 + 1 bias row, padded to 9*128
KC = KAUG // 128       # 9 contraction chunks
PAIRS = HPC // 2       # 4 head pairs per core
LT = L // 128          # 16 token tiles


def build_nc(repeats=1):
    nc = bacc.Bacc("TRN2", target_bir_lowering=False, debug=False,
                   num_devices=NCORES)

    xT = nc.dram_tensor("xT", [KAUG, L], BF16, kind="ExternalInput").ap()
    wqT = nc.dram_tensor("wqT", [KAUG, CPC], BF16, kind="ExternalInput").ap()
    wkT = nc.dram_tensor("wkT", [KAUG, CPC], BF16, kind="ExternalInput").ap()
    wvT = nc.dram_tensor("wvT", [KAUG, CPC], BF16, kind="ExternalInput").ap()
    woT = nc.dram_tensor("woT", [CPC, H], BF16, kind="ExternalInput").ap()
    maskT = nc.dram_tensor("maskT", [L, L], BF16, kind="ExternalInput").ap()
    out = nc.dram_tensor("out", [L, H], BF16, kind="ExternalOutput").ap()

    with tile.TileContext(nc) as tc:
        for _ in range(repeats):
            mhsa_body(tc, xT, wqT, wkT, wvT, woT, maskT, out)
    nc.compile()
    return nc


def mhsa_body(tc, xT, wqT, wkT, wvT, woT, maskT, out):
    nc = tc.nc
    Exp = mybir.ActivationFunctionType.Exp
    mult = mybir.AluOpType.mult

    xT_r = xT.rearrange("(kc p) t -> p kc t", p=128)
    wq_r = wqT.rearrange("(kc p) c -> p kc c", p=128)
    wk_r = wkT.rearrange("(kc p) c -> p kc c", p=128)
    wv_r = wvT.rearrange("(kc p) c -> p kc c", p=128)
    wo_r = woT.rearrange("(kc p) c -> p kc c", p=128)
    mask_r = maskT.rearrange("(jt p) i -> p jt i", p=128)
    out_r = out.rearrange("(tt p) c -> p tt c", p=128)

    ctx = contextlib.ExitStack()
    with ctx:
        wpool = ctx.enter_context(tc.tile_pool(name="weights", bufs=1))
        big = ctx.enter_context(tc.tile_pool(name="big", bufs=1))
        ppool = ctx.enter_context(tc.tile_pool(name="pm", bufs=3))
        mpool = ctx.enter_context(tc.tile_pool(name="mask", bufs=4))
        rpool = ctx.enter_context(tc.tile_pool(name="rec", bufs=2))
        opool = ctx.enter_context(tc.tile_pool(name="osb", bufs=2))
        qkv_ps = ctx.enter_context(
            tc.tile_pool(name="qkv_ps", bufs=2, space="PSUM"))
        s_ps = ctx.enter_context(
            tc.tile_pool(name="s_ps", bufs=2, space="PSUM"))
        nd_ps = ctx.enter_context(
            tc.tile_pool(name="nd_ps", bufs=1, space="PSUM"))

        wq_sb = wpool.tile([128, KC, CPC], BF16, tag="wq")
        wk_sb = wpool.tile([128, KC, CPC], BF16, tag="wk")
        wv_sb = wpool.tile([128, KC, CPC], BF16, tag="wv")
        wo_sb = wpool.tile([128, PAIRS, H], BF16, tag="wo")
        x_sb = big.tile([128, KC, L], BF16, tag="x")
        qT_sb = big.tile([128, PAIRS, L], BF16, tag="qT")
        kT_sb = big.tile([128, PAIRS, L], BF16, tag="kT")
        v_sb = big.tile([128, LT, 2 * CPC], BF16, tag="v")
        aoT_sb = big.tile([128, PAIRS, L], BF16, tag="aoT")
        pmB_sb = big.tile([128, LT, 1024], BF16, tag="pmB")

        nc.sync.dma_start(wv_sb[:], wv_r)
        for kc in range(KC):
            nc.sync.dma_start(x_sb[:, kc, :], xT_r[:, kc, :])
        nc.sync.dma_start(wq_sb[:], wq_r)
        nc.sync.dma_start(wk_sb[:], wk_r)
        nc.sync.dma_start(wo_sb[:], wo_r)

        # v layout: per head 64 ones-columns then 64 v-columns, so the AV
        # lhsT [128,128] yields denominator (out rows 0-63) and numerator
        # (rows 64-127) in one matmul.
        v_aug = v_sb[:].rearrange("p t (h two d) -> p t h two d", two=2, d=64)
        nc.any.memset(v_aug[:, :, :, 0, :], 1.0)

        def v_proj():
            for t in range(LT):
                ps = qkv_ps.tile([128, 512], F32, tag="ps")
                for kc in range(KC):
                    nc.tensor.matmul(
                        ps[:], x_sb[:, kc, t * 128:(t + 1) * 128],
                        wv_sb[:, kc, :],
                        start=(kc == 0), stop=(kc == KC - 1))
                nc.vector.tensor_copy(
                    v_aug[:, t, :, 1, :],
                    ps[:].rearrange("p (h d) -> p h d", d=64))

        def qk_proj(p):
            for nh in range(2):
                for w_sb, dst in ((wq_sb, qT_sb), (wk_sb, kT_sb)):
                    for hf in range(2):
                        c0 = nh * 1024 + hf * 512
                        ps = qkv_ps.tile([128, 512], F32, tag="ps")
                        for kc in range(KC):
                            nc.tensor.matmul(
                                ps[:], w_sb[:, kc, p * 128:(p + 1) * 128],
                                x_sb[:, kc, c0:c0 + 512],
                                start=(kc == 0), stop=(kc == KC - 1))
                        nc.vector.tensor_copy(dst[:, p, c0:c0 + 512], ps[:])

        def attention(p):
            hA, hB = 2 * p, 2 * p + 1
            for ich in range(2):
                q0 = ich * 1024
                ndA = nd_ps.tile([128, 1024], F32, tag="nd",
                                 name=f"ndA{p}_{ich}")
                for j in range(LT):
                    m_t = mpool.tile([128, 1024], BF16, tag="m")
                    nc.sync.dma_start(m_t[:], mask_r[:, j, q0:q0 + 1024])
                    sA = s_ps.tile([128, 1024], F32, tag="s",
                                   name=f"sA{p}_{ich}_{j}")
                    sB = s_ps.tile([128, 1024], F32, tag="s",
                                   name=f"sB{p}_{ich}_{j}")
                    # the pair's S matmuls adjacent: K=64 row tiles at
                    # partitions 0-63 / 64-127 run concurrently on the PE
                    for ic2 in range(2):
                        qq = q0 + ic2 * 512
                        nc.tensor.matmul(
                            sA[:, ic2 * 512:(ic2 + 1) * 512],
                            kT_sb[0:64, p, j * 128:(j + 1) * 128],
                            qT_sb[0:64, p, qq:qq + 512],
                            start=True, stop=True)
                        nc.tensor.matmul(
                            sB[:, ic2 * 512:(ic2 + 1) * 512],
                            kT_sb[64:128, p, j * 128:(j + 1) * 128],
                            qT_sb[64:128, p, qq:qq + 512],
                            start=True, stop=True)
                    pmA = ppool.tile([128, 1024], BF16, tag="pm")
                    nc.scalar.activation(pmA[:], sA[:], Exp, scale=0.125)
                    nc.scalar.activation(pmB_sb[:, j, :], sB[:], Exp,
                                         scale=0.125)
                    nc.vector.tensor_tensor(pmA[:], pmA[:], m_t[:], mult)
                    nc.vector.tensor_tensor(pmB_sb[:, j, :], pmB_sb[:, j, :],
                                            m_t[:], mult)
                    for ic2 in range(2):
                        nc.tensor.matmul(
                            ndA[:, ic2 * 512:(ic2 + 1) * 512],
                            v_sb[:, j, hA * 128:(hA + 1) * 128],
                            pmA[:, ic2 * 512:(ic2 + 1) * 512],
                            start=(j == 0), stop=(j == LT - 1))
                recA = rpool.tile([64, 1024], F32, tag="rec",
                                  name=f"recA{p}_{ich}")
                nc.vector.reciprocal_approx_fast(recA[:], ndA[0:64, :])
                nc.vector.tensor_tensor(
                    aoT_sb[0:64, p, q0:q0 + 1024], ndA[64:128, :], recA[:],
                    mult)
                ndB = nd_ps.tile([128, 1024], F32, tag="nd",
                                 name=f"ndB{p}_{ich}")
                for j in range(LT):
                    for ic2 in range(2):
                        nc.tensor.matmul(
                            ndB[:, ic2 * 512:(ic2 + 1) * 512],
                            v_sb[:, j, hB * 128:(hB + 1) * 128],
                            pmB_sb[:, j, ic2 * 512:(ic2 + 1) * 512],
                            start=(j == 0), stop=(j == LT - 1))
                recB = rpool.tile([64, 1024], F32, tag="rec",
                                  name=f"recB{p}_{ich}")
                nc.vector.reciprocal_approx_fast(recB[:], ndB[0:64, :])
                nc.vector.tensor_tensor(
                    aoT_sb[64:128, p, q0:q0 + 1024], ndB[64:128, :], recB[:],
                    mult)

        def o_proj():
            for t in range(LT):
                for hf in range(2):
                    ps = qkv_ps.tile([128, 512], F32, tag="ps")
                    for kc in range(PAIRS):
                        nc.tensor.matmul(
                            ps[:], aoT_sb[:, kc, t * 128:(t + 1) * 128],
                            wo_sb[:, kc, hf * 512:(hf + 1) * 512],
                            start=(kc == 0), stop=(kc == PAIRS - 1))
                    o_sb = opool.tile([128, 512], BF16, tag="o")
                    nc.vector.tensor_copy(o_sb[:], ps[:])
                    nc.sync.dma_start(out_r[:, t, hf * 512:(hf + 1) * 512],
                                      o_sb[:])

        qk_proj(0)
        v_proj()
        for p in range(PAIRS):
            attention(p)
            if p + 1 < PAIRS:
                qk_proj(p + 1)
        o_proj()


_NC_CACHE = None


def get_nc():
    global _NC_CACHE
    if _NC_CACHE is None:
        _NC_CACHE = build_nc()
    return _NC_CACHE


def make_in_maps(x, attn_mask, Wq, bq, Wk, bk, Wv, bv, Wo, bo):
    bf = ml_dtypes.bfloat16
    x = np.asarray(x, np.float32)
    attn_mask = np.asarray(attn_mask)
    in_maps = []
    for core in range(NCORES):
        b, pg = divmod(core, 2)
        cs = slice(pg * CPC, (pg + 1) * CPC)
        xT = np.zeros((KAUG, L), bf)
        xT[:H] = x[b].T.astype(bf)
        xT[H] = 1.0
        m = {"xT": xT}
        for name, W, bias in (("wqT", Wq, bq), ("wkT", Wk, bk),
                              ("wvT", Wv, bv)):
            wT = np.zeros((KAUG, CPC), bf)
            wT[:H] = np.asarray(W, np.float32)[cs, :].T.astype(bf)
            wT[H] = np.asarray(bias, np.float32)[cs].astype(bf)
            m[name] = wT
        m["woT"] = np.ascontiguousarray(
            np.asarray(Wo, np.float32)[:, cs].T).astype(bf)
        m["maskT"] = np.ascontiguousarray(
            attn_mask[b, 0].T).astype(bf)
        in_maps.append(m)
    return in_maps


def gather(results, bo):
    bo = np.asarray(bo, np.float32)
    out = np.empty((B, L, H), np.float32)
    for b in range(B):
        out[b] = (results[2 * b]["out"].astype(np.float32)
                  + results[2 * b + 1]["out"].astype(np.float32) + bo)
    return out


def kernel(x, attn_mask, Wq, bq, Wk, bk, Wv, bv, Wo, bo):
    nc = get_nc()
    in_maps = make_in_maps(x, attn_mask, Wq, bq, Wk, bk, Wv, bv, Wo, bo)
    res = bass_utils.run_bass_kernel_spmd(nc, in_maps,
                                          core_ids=list(range(NCORES)))
    return gather(res.results, bo)
